# revision 1
# baseline (speedup 1.0000x reference)
"""Trainium2 Bass kernel for nn_CombinedCriterionAE (retrieval 1-NN + losses).

Strategy (8 NeuronCores, SPMD), v3:
  - gt sharded along L (32768 -> 4096/core); every core holds all preds.
  - s = -dist^2 as a K=24 bf16-split matmul (tracks fp32 rounding to ~1e-6):
    the 4 N=512 matmuls of each 2048-wide chunk run CONCURRENTLY on distinct
    PE row groups via tile_position=(32t, 0) (K=24 <= 32, measured ~3x).
  - Per chunk: ACT evacuates the upper 1024 PSUM columns to SBUF, one DVE
    tensor_tensor_scan computes the running max of pairs (j, j+1024) (chained
    across chunks); ACT counts below-max prefix positions (sign + accum), so
    counts sum to the winner pair position with first-occurrence ties.
  - Index decode runs on DVE batched every 8 tiles; both candidate gt rows
    are gathered by indirect DMA INSIDE the loop (GpSimd idle there),
    hiding the old 141us gather tail.
  - Post-loop: exact fp32 d^2 resolves the pair member; per-pred loss
    contributions (d2, cos-sim) are computed locally, then one AllGather of
    [2,128,64] + a strict-less fold (keeps the earliest core = global
    first-occurrence argmin) reduces across cores. No post-collective
    gathers remain.
"""
import os
import numpy as np
import ml_dtypes

import concourse.bass as bass
import concourse.bacc as bacc
import concourse.mybir as mybir
import concourse.tile as tile
from concourse.bass import IndirectOffsetOnAxis

BF16 = ml_dtypes.bfloat16
DT = mybir.dt
OP = mybir.AluOpType
AF = mybir.ActivationFunctionType

N_PRED = 8192
L_GT = 32768
NCORES = 8
K_SMALL = 19
K_BIG = 5
KK = K_SMALL + K_BIG
NEG_INF = -3.0e38
GRP = 8  # tiles per decode group


# ----------------------------------------------------------------------------
# host-side input prep
# ----------------------------------------------------------------------------

def _split3(x):
    x = np.asarray(x, np.float32)
    hi = x.astype(BF16)
    r = x - hi.astype(np.float32)
    mid = r.astype(BF16)
    r2 = r - mid.astype(np.float32)
    lo = r2.astype(BF16)
    return hi, mid, lo


def build_operands(pred_pts, gt_pts):
    """lhsT [24, N] / rhs [24, L] bf16; 19 small rows then 5 big rows."""
    q = 2.0 * np.asarray(pred_pts, np.float32)
    qh, qm, ql = _split3(q.T)
    gh, gm, gl = _split3(np.asarray(gt_pts, np.float32).T)
    g2 = (np.asarray(gt_pts, np.float32) ** 2).sum(1)
    p2 = (np.asarray(pred_pts, np.float32) ** 2).sum(1)
    g2h, g2m, g2l = _split3(g2)
    p2h, p2m, p2l = _split3(p2)
    ones_g = np.ones(gt_pts.shape[0], BF16)
    neg1_p = -np.ones(pred_pts.shape[0], BF16)

    lhs, rhs = [], []

    def add(a, b):
        lhs.append(a)
        rhs.append(b)

    for d in range(3):
        add(qh[d], gm[d]); add(qm[d], gh[d]); add(qm[d], gm[d])
        add(qh[d], gl[d]); add(ql[d], gh[d])
    add(neg1_p, g2m); add(neg1_p, g2l)
    add((-p2m).astype(BF16), ones_g); add((-p2l).astype(BF16), ones_g)
    # big rows
    add(qh[0], gh[0]); add(qh[1], gh[1]); add(qh[2], gh[2])
    add((-p2h).astype(BF16), ones_g); add(neg1_p, g2h)
    return np.ascontiguousarray(np.stack(lhs)), np.ascontiguousarray(np.stack(rhs))


def prep_inputs(pred_feat, gt_data, n_pred, ll, ncores):
    """Returns the per-core in_map list."""
    pred_feat = np.asarray(pred_feat, np.float32)
    gt_data = np.asarray(gt_data, np.float32)
    nt = n_pred // 128
    pred_pts = pred_feat[:, :3]
    pred_nrm = pred_feat[:, 3:]
    lhsT, rhs = build_operands(pred_pts, gt_data[:, :3])

    # 4-way tile_position packing: row group t holds the same 24 lhsT rows;
    # rhs4[32t+k, 512c+v] = rhs[k, 2048c + 512t + v].
    lhs4 = np.zeros((128, n_pred), BF16)
    for t in range(4):
        lhs4[32 * t:32 * t + KK] = lhsT

    # pred arrays in [128, nt, 3] layout: element (r, i, :) = pred[i*128+r]
    pp = np.ascontiguousarray(pred_pts.reshape(nt, 128, 3).transpose(1, 0, 2))
    pn = np.ascontiguousarray(pred_nrm.reshape(nt, 128, 3).transpose(1, 0, 2))

    in_maps = []
    for c in range(ncores):
        rc = rhs[:, ll * c:ll * (c + 1)]
        rhs4 = np.zeros((128, 1024), BF16)
        for t in range(4):
            for ch in range(2):
                rhs4[32 * t:32 * t + KK, 512 * ch:512 * (ch + 1)] = \
                    rc[:, 2048 * ch + 512 * t:2048 * ch + 512 * (t + 1)]
        in_maps.append({
            "lhs4": lhs4,
            "rhs4": np.ascontiguousarray(rhs4),
            "pp": pp,
            "pn": pn,
            "cbase": np.full((128, 1), float(ll * c), np.float32),
            "gtf": gt_data,
        })
    return in_maps


# ----------------------------------------------------------------------------
# device program
# ----------------------------------------------------------------------------

def build_nc(n_pred=N_PRED, ll=L_GT // NCORES, ncores=NCORES, debug_outs=False):
    nt = n_pred // 128
    assert ll == 4096 and n_pred % 128 == 0 and nt % GRP == 0
    l_tot = ll * ncores

    nc = bacc.Bacc("TRN2", target_bir_lowering=False, debug=False,
                   num_devices=ncores)

    lhs4_d = nc.dram_tensor("lhs4", [128, n_pred], DT.bfloat16, kind="ExternalInput")
    rhs4_d = nc.dram_tensor("rhs4", [128, 1024], DT.bfloat16, kind="ExternalInput")
    pp_d = nc.dram_tensor("pp", [128, nt, 3], DT.float32, kind="ExternalInput")
    pn_d = nc.dram_tensor("pn", [128, nt, 3], DT.float32, kind="ExternalInput")
    cbase_d = nc.dram_tensor("cbase", [128, 1], DT.float32, kind="ExternalInput")
    gtf_d = nc.dram_tensor("gtf", [l_tot, 6], DT.float32, kind="ExternalInput")
    out_d = nc.dram_tensor("out", [1, 1], DT.float32, kind="ExternalOutput")
    if debug_outs:
        dbg_pp_d = nc.dram_tensor("dbg_pp", [128, nt], DT.float32, kind="ExternalOutput")
        dbg_i0_d = nc.dram_tensor("dbg_i0", [128, nt], DT.float32, kind="ExternalOutput")
        dbg_d2_d = nc.dram_tensor("dbg_d2", [128, nt], DT.float32, kind="ExternalOutput")
        dbg_cl_d = nc.dram_tensor("dbg_cl", [128, nt], DT.float32, kind="ExternalOutput")

    with tile.TileContext(nc) as tc:
        with (
            tc.tile_pool(name="persist", bufs=1) as pers,
            tc.tile_pool(name="hpool", bufs=4) as hpool,
            tc.tile_pool(name="scnpool", bufs=4) as scnpool,
            tc.tile_pool(name="mkpool", bufs=4) as mkpool,
            tc.tile_pool(name="dpool", bufs=3) as dpool,
            tc.tile_pool(name="dram", bufs=1, space="DRAM") as dram,
        ):
            # ---- persistent SBUF loads -------------------------------------
            LHS4 = pers.tile([128, n_pred], DT.bfloat16)
            RHS4 = pers.tile([128, 1024], DT.bfloat16)
            PP = pers.tile([128, nt, 3], DT.float32)
            PN = pers.tile([128, nt, 3], DT.float32)
            CBASE = pers.tile([128, 1], DT.float32)
            nc.sync.dma_start(LHS4[:], lhs4_d[:])
            nc.sync.dma_start(RHS4[:], rhs4_d[:])
            nc.sync.dma_start(PP[:], pp_d[:])
            nc.sync.dma_start(PN[:], pn_d[:])
            nc.sync.dma_start(CBASE[:], cbase_d[:])

            CNT0 = pers.tile([128, nt], DT.float32)
            CNT1 = pers.tile([128, nt], DT.float32)
            I0T = pers.tile([128, nt], DT.int32)
            I1T = pers.tile([128, nt], DT.int32)
            G0 = pers.tile([128, nt, 6], DT.float32)
            G1 = pers.tile([128, nt, 6], DT.float32)
            if debug_outs:
                DBG_PP = pers.tile([128, nt], DT.float32)
                DBG_I0 = pers.tile([128, nt], DT.float32)

            # ---- main loop -------------------------------------------------
            with tc.tile_pool(name="spsum", bufs=2, space="PSUM") as spsum:
                for i in range(nt):
                    scn_tiles = []
                    for c in range(2):
                        P = spsum.tile([128, 2048], DT.float32, tag="P")
                        for t in range(4):
                            nc.tensor.matmul(
                                P[:, 512 * t:512 * (t + 1)],
                                LHS4[32 * t:32 * t + KK, 128 * i:128 * (i + 1)],
                                RHS4[32 * t:32 * t + KK, 512 * c:512 * (c + 1)],
                                start=True, stop=True,
                                tile_position=(32 * t, 0),
                            )
                        HB = hpool.tile([128, 1024], DT.float32, tag="HB")
                        nc.scalar.activation(
                            out=HB[:], in_=P[:, 1024:2048], func=AF.Copy,
                        )
                        # absorb the PE wait into a tiny copy (the scan's ISA
                        # struct has few sync-wait slots)
                        FEN = hpool.tile([128, 1], DT.float32, tag="FEN")
                        nc.vector.tensor_copy(out=FEN[:, 0:1], in_=P[:, 0:1])
                        SCN = scnpool.tile([128, 1024], DT.float32, tag="SCN")
                        nc.vector.tensor_tensor_scan(
                            out=SCN[:],
                            data0=P[:, 0:1024],
                            data1=HB[:],
                            initial=NEG_INF if c == 0 else scn_tiles[-1][:, 1023:1024],
                            op0=OP.max,
                            op1=OP.max,
                        )
                        scn_tiles.append(SCN)
                    smax_ap = scn_tiles[-1][:, 1023:1024]
                    cnts = [CNT0, CNT1]
                    for c in range(2):
                        # ACT counts below-max prefix positions: the count sum
                        # IS the winner pair position (first-occurrence ties).
                        MK = mkpool.tile([128, 1024], DT.float16, tag="MK")
                        nc.scalar.activation(
                            out=MK[:], in_=scn_tiles[c][:],
                            func=AF.Sign,
                            bias=smax_ap, scale=-1.0,
                            accum_out=cnts[c][:, i:i + 1],
                        )

                    # ---- batched decode + gathers every GRP tiles ----------
                    if i % GRP == GRP - 1:
                        g0 = i - (GRP - 1)
                        sl = slice(g0, i + 1)
                        PPOS = dpool.tile([128, GRP], DT.float32, tag="PPOS")
                        CF = dpool.tile([128, GRP], DT.float32, tag="CF")
                        L0G = dpool.tile([128, GRP], DT.float32, tag="L0G")
                        L1G = dpool.tile([128, GRP], DT.float32, tag="L1G")
                        nc.vector.tensor_tensor(out=PPOS[:], in0=CNT0[:, sl],
                                                in1=CNT1[:, sl], op=OP.add)
                        nc.vector.tensor_scalar(
                            out=CF[:], in0=PPOS[:],
                            scalar1=1024.0, scalar2=1024.0, op0=OP.is_ge, op1=OP.mult,
                        )
                        nc.vector.tensor_tensor(out=L0G[:], in0=PPOS[:], in1=CF[:], op=OP.add)
                        nc.vector.tensor_scalar(
                            out=L0G[:], in0=L0G[:], scalar1=CBASE[:, 0:1], scalar2=None,
                            op0=OP.add,
                        )
                        nc.vector.tensor_scalar(
                            out=L1G[:], in0=L0G[:], scalar1=1024.0, scalar2=None,
                            op0=OP.add,
                        )
                        nc.vector.tensor_copy(out=I0T[:, sl], in_=L0G[:])
                        nc.vector.tensor_copy(out=I1T[:, sl], in_=L1G[:])
                        if debug_outs:
                            nc.vector.tensor_copy(out=DBG_PP[:, sl], in_=PPOS[:])
                            nc.vector.tensor_copy(out=DBG_I0[:, sl], in_=L0G[:])
                        for j in range(g0, i + 1):
                            nc.gpsimd.indirect_dma_start(
                                out=G0[:, j, :], out_offset=None, in_=gtf_d[:],
                                in_offset=IndirectOffsetOnAxis(ap=I0T[:, j:j + 1], axis=0),
                            )
                            nc.gpsimd.indirect_dma_start(
                                out=G1[:, j, :], out_offset=None, in_=gtf_d[:],
                                in_offset=IndirectOffsetOnAxis(ap=I1T[:, j:j + 1], axis=0),
                            )

            # ---- resolve pair member with exact fp32 distances --------------
            DF = pers.tile([128, nt, 3], DT.float32)
            SQ = pers.tile([128, nt, 3], DT.float32)
            D0 = pers.tile([128, nt], DT.float32)
            D1 = pers.tile([128, nt], DT.float32)
            nc.vector.tensor_tensor(out=DF[:], in0=PP[:], in1=G0[:, :, 0:3], op=OP.subtract)
            nc.vector.tensor_tensor(out=SQ[:], in0=DF[:], in1=DF[:], op=OP.mult)
            nc.vector.tensor_reduce(out=D0[:], in_=SQ[:], axis=mybir.AxisListType.X, op=OP.add)
            nc.vector.tensor_tensor(out=DF[:], in0=PP[:], in1=G1[:, :, 0:3], op=OP.subtract)
            nc.vector.tensor_tensor(out=SQ[:], in0=DF[:], in1=DF[:], op=OP.mult)
            nc.vector.tensor_reduce(out=D1[:], in_=SQ[:], axis=mybir.AxisListType.X, op=OP.add)
            MEM = pers.tile([128, nt], DT.uint8)
            nc.vector.tensor_tensor(out=MEM[:], in0=D1[:], in1=D0[:], op=OP.is_ge)
            D2 = pers.tile([128, nt], DT.float32)
            nc.vector.tensor_tensor(out=D2[:], in0=D0[:], in1=D1[:], op=OP.min)
            MNR = pers.tile([128, nt, 3], DT.float32)
            for d in range(3, 6):
                nc.vector.select(out=MNR[:, :, d - 3], mask=MEM[:],
                                 on_true=G0[:, :, d], on_false=G1[:, :, d])

            # ---- per-pred cos similarity -----------------------------------
            def normalize(src3, dst3, tagp):
                NSQ = pers.tile([128, nt, 3], DT.float32, tag=f"NSQ{tagp}", name=f"NSQ{tagp}")
                NS = pers.tile([128, nt], DT.float32, tag=f"NS{tagp}", name=f"NS{tagp}")
                nc.vector.tensor_tensor(out=NSQ[:], in0=src3, in1=src3, op=OP.mult)
                nc.vector.tensor_reduce(out=NS[:], in_=NSQ[:], axis=mybir.AxisListType.X, op=OP.add)
                nc.scalar.activation(out=NS[:], in_=NS[:], func=AF.Sqrt)
                nc.vector.tensor_scalar(out=NS[:], in0=NS[:], scalar1=1e-4,
                                        scalar2=None, op0=OP.max)
                nc.vector.reciprocal(out=NS[:], in_=NS[:])
                for d in range(3):
                    nc.vector.tensor_tensor(out=dst3[:, :, d], in0=src3[:, :, d],
                                            in1=NS[:], op=OP.mult)

            PNH = pers.tile([128, nt, 3], DT.float32)
            MNH = pers.tile([128, nt, 3], DT.float32)
            normalize(PN[:], PNH, "a")
            normalize(MNR[:], MNH, "b")
            CC3 = pers.tile([128, nt, 3], DT.float32)
            CL = pers.tile([128, nt], DT.float32)
            nc.vector.tensor_tensor(out=CC3[:], in0=PNH[:], in1=MNH[:], op=OP.mult)
            nc.vector.tensor_reduce(out=CL[:], in_=CC3[:], axis=mybir.AxisListType.X, op=OP.add)

            # ---- AllGather (d2, cos) across cores --------------------------
            cc_in = dram.tile([2, 128, nt], DT.float32)
            cc_out = dram.tile([ncores, 2, 128, nt], DT.float32, addr_space="Shared")
            nc.sync.dma_start(cc_in[0], D2[:])
            nc.sync.dma_start(cc_in[1], CL[:])
            nc.gpsimd.collective_compute(
                "AllGather",
                OP.bypass,
                replica_groups=[list(range(ncores))],
                ins=[cc_in[:].opt()],
                outs=[cc_out[:].opt()],
            )

            # ---- fold cores (strict-less keeps earliest core) --------------
            RUNV = pers.tile([128, nt], DT.float32)
            RUNL = pers.tile([128, nt], DT.float32)
            nc.sync.dma_start(RUNV[:], cc_out[0, 0])
            nc.sync.dma_start(RUNL[:], cc_out[0, 1])
            with tc.tile_pool(name="fold", bufs=2) as fold:
                for j in range(1, ncores):
                    VJ = fold.tile([128, nt], DT.float32, tag="VJ")
                    LJ = fold.tile([128, nt], DT.float32, tag="LJ")
                    nc.sync.dma_start(VJ[:], cc_out[j, 0])
                    nc.sync.dma_start(LJ[:], cc_out[j, 1])
                    CM = fold.tile([128, nt], DT.uint8, tag="CM")
                    nc.vector.tensor_tensor(out=CM[:], in0=VJ[:], in1=RUNV[:], op=OP.is_lt)
                    NV = fold.tile([128, nt], DT.float32, tag="NV")
                    NL = fold.tile([128, nt], DT.float32, tag="NL")
                    nc.vector.tensor_tensor(out=NV[:], in0=VJ[:], in1=RUNV[:], op=OP.min)
                    nc.vector.select(out=NL[:], mask=CM[:], on_true=LJ[:], on_false=RUNL[:])
                    RUNV, RUNL = NV, NL
            if debug_outs:
                nc.sync.dma_start(dbg_pp_d[:], DBG_PP[:])
                nc.sync.dma_start(dbg_i0_d[:], DBG_I0[:])
                nc.sync.dma_start(dbg_d2_d[:], RUNV[:])
                nc.sync.dma_start(dbg_cl_d[:], RUNL[:])

            # ---- scalar losses ---------------------------------------------
            ILS = pers.tile([128, 1], DT.float32)
            CSUM = pers.tile([128, 1], DT.float32)
            nc.vector.tensor_reduce(out=ILS[:], in_=RUNV[:],
                                    axis=mybir.AxisListType.X, op=OP.add)
            nc.vector.tensor_reduce(out=CSUM[:], in_=RUNL[:],
                                    axis=mybir.AxisListType.X, op=OP.add)

            # partition-sum via ones-matmul, then the final scalar
            SUM2 = pers.tile([128, 2], DT.float32)
            ONES = pers.tile([128, 1], DT.float32)
            nc.vector.memset(ONES[:], 1.0)
            nc.vector.tensor_copy(out=SUM2[:, 0:1], in_=ILS[:])
            nc.vector.tensor_copy(out=SUM2[:, 1:2], in_=CSUM[:])
            with tc.tile_pool(name="fpsum", bufs=1, space="PSUM") as fpsum:
                SP = fpsum.tile([1, 2], DT.float32)
                nc.tensor.matmul(SP[:], ONES[:], SUM2[:], start=True, stop=True)
                FIN = pers.tile([1, 2], DT.float32)
                nc.vector.tensor_copy(out=FIN[:], in_=SP[:])
            A = pers.tile([1, 1], DT.float32)
            B = pers.tile([1, 1], DT.float32)
            OUTS = pers.tile([1, 1], DT.float32)
            nc.vector.tensor_scalar(out=A[:], in0=FIN[0:1, 0:1],
                                    scalar1=1.0 / (n_pred * 3), scalar2=None, op0=OP.mult)
            nc.vector.tensor_scalar(out=B[:], in0=FIN[0:1, 1:2],
                                    scalar1=1.0 / n_pred, scalar2=None, op0=OP.mult)
            nc.vector.tensor_tensor(out=OUTS[:], in0=A[:], in1=B[:], op=OP.subtract)
            nc.vector.tensor_scalar(out=OUTS[:], in0=OUTS[:], scalar1=1.0,
                                    scalar2=None, op0=OP.add)
            nc.sync.dma_start(out_d[:], OUTS[:])

    nc.compile()
    return nc


# ----------------------------------------------------------------------------
# public entry point
# ----------------------------------------------------------------------------

_CACHED_NC = None


def kernel(pred_feat, pred_decoder, input_data, gt_data):
    global _CACHED_NC
    from concourse.bass_utils import run_bass_kernel_spmd

    ll = L_GT // NCORES
    in_maps = prep_inputs(pred_feat, gt_data, N_PRED, ll, NCORES)
    if _CACHED_NC is None:
        _CACHED_NC = build_nc(N_PRED, ll, NCORES,
                              debug_outs=bool(int(os.environ.get("KERNEL_DEBUG", "0"))))
    res = run_bass_kernel_spmd(_CACHED_NC, in_maps, list(range(NCORES)),
                               trace=bool(int(os.environ.get("KERNEL_TRACE", "0"))))
    out = np.asarray(res.results[0]["out"], np.float32).reshape(())
    kernel.last_results = res
    return out



# revision 8
# speedup vs baseline: 1.1318x; 1.1318x over previous
"""Trainium2 Bass kernel for nn_CombinedCriterionAE (retrieval 1-NN + losses).

Strategy (8 NeuronCores, SPMD), v6:
  - gt sharded along L (32768 -> 4096/core); every core holds all preds.
  - s = -dist^2 as a K=24 bf16-split matmul (tracks fp32 rounding to ~1e-6).
  - Per pred-tile (128 preds x 4096 gt), columns are paired (l, l+2048) and
    consumed in two steps s=0,1. Step s computes C(s) = cols [1024s, 1024s+1024)
    and C(s+2) = cols [2048+1024s, ...) as 4 concurrent 512-col matmuls on
    distinct PE row quadrants (tile_position): quadrants 0/1 -> Pa (= C(s)),
    quadrants 2/3 -> Pb (= C(s+2)). ONE ACT copy evacuates Pa to SBUF; the
    DVE scan (data0=Pb PSUM, data1=Pa-copy) emits the running max of pairs,
    chained across the two steps -> monotone SCN [128, 2048].
  - ONE ACT Sign+accum over the ODD scan positions (1024 strided reads):
    count c -> winner pair-position p* in {2c, 2c+1}. Candidate gt rows
    {a, a+1} u {a+2048, a+2049} with a = min(2c, 2046): two 2-consecutive-row
    indirect gathers per tile (descriptor-count bound, same cost as 1-row).
  - Sign counts are software-pipelined one tile behind so copies always
    precede signs in ACT program order (no scan-chain stalls).
  - PSUM: two pools (copied / scanned) of 2 banks x 2 bufs = 8 banks total,
    fully double-buffered.
  - Post-loop: exact fp32 d^2 over the 4 candidates folded in index order
    (first-occurrence ties match the reference), AllReduce-min over d2,
    equality-masked cos, scalar AllReduce-sum for the cos term.
"""
import os
import numpy as np
import ml_dtypes

import concourse.bass as bass
import concourse.bacc as bacc
import concourse.mybir as mybir
import concourse.tile as tile
from concourse.bass import IndirectOffsetOnAxis

BF16 = ml_dtypes.bfloat16
DT = mybir.dt
OP = mybir.AluOpType
AF = mybir.ActivationFunctionType

N_PRED = 8192
L_GT = 32768
NCORES = 8
K_SMALL = 19
K_BIG = 5
KK = K_SMALL + K_BIG
NEG_INF = -3.0e38
GRP = 8  # tiles per decode group
NLHS = 4  # LHS split for earlier loop start


# ----------------------------------------------------------------------------
# host-side input prep
# ----------------------------------------------------------------------------

def _split3(x):
    x = np.asarray(x, np.float32)
    hi = x.astype(BF16)
    r = x - hi.astype(np.float32)
    mid = r.astype(BF16)
    r2 = r - mid.astype(np.float32)
    lo = r2.astype(BF16)
    return hi, mid, lo


def build_operands(pred_pts, gt_pts):
    """lhsT [24, N] / rhs [24, L] bf16; 19 small rows then 5 big rows."""
    q = 2.0 * np.asarray(pred_pts, np.float32)
    qh, qm, ql = _split3(q.T)
    gh, gm, gl = _split3(np.asarray(gt_pts, np.float32).T)
    g2 = (np.asarray(gt_pts, np.float32) ** 2).sum(1)
    p2 = (np.asarray(pred_pts, np.float32) ** 2).sum(1)
    g2h, g2m, g2l = _split3(g2)
    p2h, p2m, p2l = _split3(p2)
    ones_g = np.ones(gt_pts.shape[0], BF16)
    neg1_p = -np.ones(pred_pts.shape[0], BF16)

    lhs, rhs = [], []

    def add(a, b):
        lhs.append(a)
        rhs.append(b)

    for d in range(3):
        add(qh[d], gm[d]); add(qm[d], gh[d]); add(qm[d], gm[d])
        add(qh[d], gl[d]); add(ql[d], gh[d])
    add(neg1_p, g2m); add(neg1_p, g2l)
    add((-p2m).astype(BF16), ones_g); add((-p2l).astype(BF16), ones_g)
    # big rows
    add(qh[0], gh[0]); add(qh[1], gh[1]); add(qh[2], gh[2])
    add((-p2h).astype(BF16), ones_g); add(neg1_p, g2h)
    return np.ascontiguousarray(np.stack(lhs)), np.ascontiguousarray(np.stack(rhs))


def prep_inputs(pred_feat, gt_data, n_pred, ll, ncores):
    """Returns the per-core in_map list."""
    pred_feat = np.asarray(pred_feat, np.float32)
    gt_data = np.asarray(gt_data, np.float32)
    nt = n_pred // 128
    pred_pts = pred_feat[:, :3]
    pred_nrm = pred_feat[:, 3:]
    lhsT, rhs = build_operands(pred_pts, gt_data[:, :3])

    # 4-way tile_position packing: row quadrant q holds the same 24 lhsT rows.
    lhs4 = np.zeros((128, n_pred), BF16)
    for q in range(4):
        lhs4[32 * q:32 * q + KK] = lhsT

    # pred arrays in [128, nt, 3] layout: element (r, i, :) = pred[i*128+r]
    pp = np.ascontiguousarray(pred_pts.reshape(nt, 128, 3).transpose(1, 0, 2))
    pn = np.ascontiguousarray(pred_nrm.reshape(nt, 128, 3).transpose(1, 0, 2))

    in_maps = []
    for c in range(ncores):
        rc = rhs[:, ll * c:ll * (c + 1)]
        # rhs4[32q+k, 512s+v]: quadrant q, step s:
        #   col0 = 2048*(q//2) + 1024*s + 512*(q%2)
        rhs4 = np.zeros((128, 1024), BF16)
        for q in range(4):
            for s in range(2):
                col0 = 2048 * (q // 2) + 1024 * s + 512 * (q % 2)
                rhs4[32 * q:32 * q + KK, 512 * s:512 * (s + 1)] = \
                    rc[:, col0:col0 + 512]
        in_maps.append({
            "rhs4": np.ascontiguousarray(rhs4),
            **{f"lhs4_{j}": np.ascontiguousarray(
                lhs4[:, j * (n_pred // NLHS):(j + 1) * (n_pred // NLHS)])
               for j in range(NLHS)},
            "pp": pp,
            "pn": pn,
            "cbase": np.full((128, 1), float(ll * c), np.float32),
            "gtf": gt_data,
        })
    return in_maps


# ----------------------------------------------------------------------------
# device program
# ----------------------------------------------------------------------------

def build_nc(n_pred=N_PRED, ll=L_GT // NCORES, ncores=NCORES, debug_outs=False):
    nt = n_pred // 128
    assert ll == 4096 and n_pred % 128 == 0 and nt % GRP == 0
    l_tot = ll * ncores
    nsub = n_pred // NLHS
    tsub = nsub // 128  # pred-tiles per LHS subtile

    nc = bacc.Bacc("TRN2", target_bir_lowering=False, debug=False,
                   num_devices=ncores)

    lhs_d = [nc.dram_tensor(f"lhs4_{j}", [128, nsub], DT.bfloat16,
                            kind="ExternalInput") for j in range(NLHS)]
    rhs4_d = nc.dram_tensor("rhs4", [128, 1024], DT.bfloat16, kind="ExternalInput")
    pp_d = nc.dram_tensor("pp", [128, nt, 3], DT.float32, kind="ExternalInput")
    pn_d = nc.dram_tensor("pn", [128, nt, 3], DT.float32, kind="ExternalInput")
    cbase_d = nc.dram_tensor("cbase", [128, 1], DT.float32, kind="ExternalInput")
    gtf_d = nc.dram_tensor("gtf", [l_tot, 6], DT.float32, kind="ExternalInput")
    out_d = nc.dram_tensor("out", [1, 1], DT.float32, kind="ExternalOutput")
    if debug_outs:
        dbg_cnt_d = nc.dram_tensor("dbg_cnt", [128, nt], DT.float32, kind="ExternalOutput")
        dbg_i0_d = nc.dram_tensor("dbg_i0", [128, nt], DT.float32, kind="ExternalOutput")
        dbg_d2_d = nc.dram_tensor("dbg_d2", [128, nt], DT.float32, kind="ExternalOutput")
        dbg_cl_d = nc.dram_tensor("dbg_cl", [128, nt], DT.float32, kind="ExternalOutput")

    with tile.TileContext(nc) as tc:
        with (
            tc.tile_pool(name="persist", bufs=1) as pers,
            tc.tile_pool(name="hpool", bufs=4) as hpool,
            tc.tile_pool(name="scnpool", bufs=3) as scnpool,
            tc.tile_pool(name="mkpool", bufs=2) as mkpool,
            tc.tile_pool(name="dpool", bufs=3) as dpool,
            tc.tile_pool(name="dram", bufs=1, space="DRAM") as dram,
        ):
            # ---- persistent SBUF loads -------------------------------------
            RHS4 = pers.tile([128, 1024], DT.bfloat16)
            nc.sync.dma_start(RHS4[:], rhs4_d[:])
            LHS = []
            for j in range(NLHS):
                L = pers.tile([128, nsub], DT.bfloat16, name=f"LHS{j}")
                nc.sync.dma_start(L[:], lhs_d[j][:])
                LHS.append(L)
            PP = pers.tile([128, nt, 3], DT.float32)
            PN = pers.tile([128, nt, 3], DT.float32)
            CBASE = pers.tile([128, 1], DT.float32)
            nc.sync.dma_start(PP[:], pp_d[:])
            nc.sync.dma_start(PN[:], pn_d[:])
            nc.sync.dma_start(CBASE[:], cbase_d[:])

            CNT = pers.tile([128, nt], DT.float32)
            I0T = pers.tile([128, nt], DT.int32)
            I1T = pers.tile([128, nt], DT.int32)
            GA = pers.tile([128, nt, 2, 6], DT.float32)
            GB = pers.tile([128, nt, 2, 6], DT.float32)

            # ---- main loop (signs pipelined one tile behind) ----------------
            with (
                tc.tile_pool(name="psa", bufs=2, space="PSUM") as psa,
                tc.tile_pool(name="psb", bufs=2, space="PSUM") as psb,
            ):
                scn_tiles = {}

                def emit_compute(i):
                    lhs_t = LHS[i // tsub]
                    lo = 128 * (i % tsub)
                    SCN = scnpool.tile([128, 2048], DT.float32, tag="SCN")
                    for s in range(2):
                        Pa = psa.tile([128, 1024], DT.float32, tag="Pa")
                        Pb = psb.tile([128, 1024], DT.float32, tag="Pb")
                        for q in range(4):
                            P = Pa if q < 2 else Pb
                            h = q % 2
                            nc.tensor.matmul(
                                P[:, 512 * h:512 * (h + 1)],
                                lhs_t[32 * q:32 * q + KK, lo:lo + 128],
                                RHS4[32 * q:32 * q + KK, 512 * s:512 * (s + 1)],
                                start=True, stop=True,
                                tile_position=(32 * q, 0),
                            )
                        HB = hpool.tile([128, 1024], DT.float32, tag="HB")
                        nc.scalar.activation(out=HB[:], in_=Pa[:], func=AF.Copy)
                        # absorb the PE sem waits into a tiny copy (the scan's
                        # ISA struct has few sync-wait slots)
                        FEN = hpool.tile([128, 1], DT.float32, tag="FEN")
                        nc.vector.tensor_copy(out=FEN[:, 0:1], in_=Pb[:, 0:1])
                        nc.vector.tensor_tensor_scan(
                            out=SCN[:, 1024 * s:1024 * (s + 1)],
                            data0=Pb[:],
                            data1=HB[:],
                            initial=NEG_INF if s == 0 else SCN[:, 1023:1024],
                            op0=OP.max,
                            op1=OP.max,
                        )
                    scn_tiles[i] = SCN

                def emit_count(i):
                    SCN = scn_tiles.pop(i)
                    MK = mkpool.tile([128, 1024], DT.float16, tag="MK")
                    # count strided (odd) positions strictly below the max
                    nc.scalar.activation(
                        out=MK[:], in_=SCN[:, 1::2],
                        func=AF.Sign,
                        bias=SCN[:, 2047:2048],
                        scale=-1.0,
                        accum_out=CNT[:, i:i + 1],
                    )

                def emit_decode(g0, g1):
                    sl = slice(g0, g1)
                    n = g1 - g0
                    A = dpool.tile([128, GRP], DT.float32, tag="A")
                    B = dpool.tile([128, GRP], DT.float32, tag="B")
                    # a = min(2c, 2046) + cbase  (p* in {2c, 2c+1})
                    nc.vector.tensor_scalar(
                        out=A[:, :n], in0=CNT[:, sl],
                        scalar1=2.0, scalar2=2046.0, op0=OP.mult, op1=OP.min,
                    )
                    nc.vector.tensor_scalar(
                        out=A[:, :n], in0=A[:, :n],
                        scalar1=CBASE[:, 0:1], scalar2=None, op0=OP.add,
                    )
                    nc.vector.tensor_scalar(
                        out=B[:, :n], in0=A[:, :n],
                        scalar1=2048.0, scalar2=None, op0=OP.add,
                    )
                    nc.vector.tensor_copy(out=I0T[:, sl], in_=A[:, :n])
                    nc.vector.tensor_copy(out=I1T[:, sl], in_=B[:, :n])
                    for j in range(g0, g1):
                        nc.gpsimd.indirect_dma_start(
                            out=GA[:, j, :, :], out_offset=None, in_=gtf_d[:],
                            in_offset=IndirectOffsetOnAxis(ap=I0T[:, j:j + 1], axis=0),
                        )
                        nc.gpsimd.indirect_dma_start(
                            out=GB[:, j, :, :], out_offset=None, in_=gtf_d[:],
                            in_offset=IndirectOffsetOnAxis(ap=I1T[:, j:j + 1], axis=0),
                        )

                for i in range(nt):
                    emit_compute(i)
                    if i >= 1:
                        emit_count(i - 1)
                        if (i - 1) % GRP == GRP - 1:
                            emit_decode(i - GRP, i)
                emit_count(nt - 1)
                emit_decode(nt - GRP, nt)

            # ---- exact fp32 resolve of the 4 candidates ---------------------
            # fold in index order a < a+1 < a+2048 < a+2049 (first-occurrence
            # ties match the reference argmin)
            DF = pers.tile([128, nt, 3], DT.float32)
            SQ = pers.tile([128, nt, 3], DT.float32)
            DCAND = [pers.tile([128, nt], DT.float32, name=f"DC{k}") for k in range(4)]
            for k in range(4):
                G = (GA, GA, GB, GB)[k]
                m = k % 2
                nc.vector.tensor_tensor(out=DF[:], in0=PP[:], in1=G[:, :, m, 0:3],
                                        op=OP.subtract)
                nc.vector.tensor_tensor(out=SQ[:], in0=DF[:], in1=DF[:], op=OP.mult)
                nc.vector.tensor_reduce(out=DCAND[k][:], in_=SQ[:],
                                        axis=mybir.AxisListType.X, op=OP.add)

            D2 = pers.tile([128, nt], DT.float32)
            MNR = pers.tile([128, nt, 3], DT.float32)
            CM = pers.tile([128, nt], DT.uint8)
            nc.vector.tensor_copy(out=D2[:], in_=DCAND[0][:])
            nc.vector.tensor_copy(out=MNR[:], in_=GA[:, :, 0, 3:6])
            for k in range(1, 4):
                G = (GA, GA, GB, GB)[k]
                m = k % 2
                nc.vector.tensor_tensor(out=CM[:], in0=DCAND[k][:], in1=D2[:],
                                        op=OP.is_lt)
                nc.vector.tensor_tensor(out=D2[:], in0=DCAND[k][:], in1=D2[:],
                                        op=OP.min)
                for d in range(3):
                    nc.vector.select(out=MNR[:, :, d], mask=CM[:],
                                     on_true=G[:, :, m, 3 + d], on_false=MNR[:, :, d])

            # ---- per-pred cos similarity -----------------------------------
            def normalize(src3, dst3, tagp):
                NSQ = pers.tile([128, nt, 3], DT.float32, tag=f"NSQ{tagp}", name=f"NSQ{tagp}")
                NS = pers.tile([128, nt], DT.float32, tag=f"NS{tagp}", name=f"NS{tagp}")
                nc.vector.tensor_tensor(out=NSQ[:], in0=src3, in1=src3, op=OP.mult)
                nc.vector.tensor_reduce(out=NS[:], in_=NSQ[:], axis=mybir.AxisListType.X, op=OP.add)
                nc.scalar.activation(out=NS[:], in_=NS[:], func=AF.Sqrt)
                nc.vector.tensor_scalar(out=NS[:], in0=NS[:], scalar1=1e-4,
                                        scalar2=None, op0=OP.max)
                nc.vector.reciprocal(out=NS[:], in_=NS[:])
                for d in range(3):
                    nc.vector.tensor_tensor(out=dst3[:, :, d], in0=src3[:, :, d],
                                            in1=NS[:], op=OP.mult)

            PNH = pers.tile([128, nt, 3], DT.float32)
            MNH = pers.tile([128, nt, 3], DT.float32)
            normalize(PN[:], PNH, "a")
            normalize(MNR[:], MNH, "b")
            CC3 = pers.tile([128, nt, 3], DT.float32)
            CL = pers.tile([128, nt], DT.float32)
            nc.vector.tensor_tensor(out=CC3[:], in0=PNH[:], in1=MNH[:], op=OP.mult)
            nc.vector.tensor_reduce(out=CL[:], in_=CC3[:], axis=mybir.AxisListType.X, op=OP.add)

            # ---- AllReduce-min over d2, equality-masked cos -----------------
            cc_in = dram.tile([128, nt], DT.float32)
            cc_out = dram.tile([128, nt], DT.float32, addr_space="Shared")
            nc.sync.dma_start(cc_in[:], D2[:])
            nc.gpsimd.collective_compute(
                "AllReduce",
                OP.min,
                replica_groups=[list(range(ncores))],
                ins=[cc_in[:].opt()],
                outs=[cc_out[:].opt()],
            )
            D2G = pers.tile([128, nt], DT.float32)
            nc.sync.dma_start(D2G[:], cc_out[:])
            MSK = pers.tile([128, nt], DT.uint8)
            CLM = pers.tile([128, nt], DT.float32)
            nc.vector.tensor_tensor(out=MSK[:], in0=D2[:], in1=D2G[:], op=OP.is_equal)
            nc.vector.memset(CLM[:], 0.0)
            nc.vector.copy_predicated(out=CLM[:], mask=MSK[:], data=CL[:])
            if debug_outs:
                nc.sync.dma_start(dbg_cnt_d[:], CNT[:])
                DBG_I0 = pers.tile([128, nt], DT.float32)
                nc.vector.tensor_copy(out=DBG_I0[:], in_=I0T[:])
                nc.sync.dma_start(dbg_i0_d[:], DBG_I0[:])
                nc.sync.dma_start(dbg_d2_d[:], D2G[:])
                nc.sync.dma_start(dbg_cl_d[:], CLM[:])

            # ---- scalar losses ---------------------------------------------
            ILS = pers.tile([128, 1], DT.float32)
            CSUM = pers.tile([128, 1], DT.float32)
            nc.vector.tensor_reduce(out=ILS[:], in_=D2G[:],
                                    axis=mybir.AxisListType.X, op=OP.add)
            nc.vector.tensor_reduce(out=CSUM[:], in_=CLM[:],
                                    axis=mybir.AxisListType.X, op=OP.add)

            # partition-sum via ones-matmul
            SUM2 = pers.tile([128, 2], DT.float32)
            ONES = pers.tile([128, 1], DT.float32)
            nc.vector.memset(ONES[:], 1.0)
            nc.vector.tensor_copy(out=SUM2[:, 0:1], in_=ILS[:])
            nc.vector.tensor_copy(out=SUM2[:, 1:2], in_=CSUM[:])
            with tc.tile_pool(name="fpsum", bufs=1, space="PSUM") as fpsum:
                SP = fpsum.tile([1, 2], DT.float32)
                nc.tensor.matmul(SP[:], ONES[:], SUM2[:], start=True, stop=True)
                FIN = pers.tile([1, 2], DT.float32)
                nc.vector.tensor_copy(out=FIN[:], in_=SP[:])

            # cos partial sums must be AllReduce-summed (the inlier sum is
            # already globally identical after the min-AllReduce)
            cs_in = dram.tile([1, 1], DT.float32)
            cs_out = dram.tile([1, 1], DT.float32, addr_space="Shared")
            nc.sync.dma_start(cs_in[:], FIN[0:1, 1:2])
            nc.gpsimd.collective_compute(
                "AllReduce",
                OP.add,
                replica_groups=[list(range(ncores))],
                ins=[cs_in[:].opt()],
                outs=[cs_out[:].opt()],
            )
            CSG = pers.tile([1, 1], DT.float32)
            nc.sync.dma_start(CSG[:], cs_out[:])

            A1 = pers.tile([1, 1], DT.float32)
            B1 = pers.tile([1, 1], DT.float32)
            OUTS = pers.tile([1, 1], DT.float32)
            nc.vector.tensor_scalar(out=A1[:], in0=FIN[0:1, 0:1],
                                    scalar1=1.0 / (n_pred * 3), scalar2=None, op0=OP.mult)
            nc.vector.tensor_scalar(out=B1[:], in0=CSG[:],
                                    scalar1=1.0 / n_pred, scalar2=None, op0=OP.mult)
            nc.vector.tensor_tensor(out=OUTS[:], in0=A1[:], in1=B1[:], op=OP.subtract)
            nc.vector.tensor_scalar(out=OUTS[:], in0=OUTS[:], scalar1=1.0,
                                    scalar2=None, op0=OP.add)
            nc.sync.dma_start(out_d[:], OUTS[:])

    nc.compile()
    return nc


# ----------------------------------------------------------------------------
# public entry point
# ----------------------------------------------------------------------------

_CACHED_NC = None


def kernel(pred_feat, pred_decoder, input_data, gt_data):
    global _CACHED_NC
    from concourse.bass_utils import run_bass_kernel_spmd

    ll = L_GT // NCORES
    in_maps = prep_inputs(pred_feat, gt_data, N_PRED, ll, NCORES)
    if _CACHED_NC is None:
        _CACHED_NC = build_nc(N_PRED, ll, NCORES,
                              debug_outs=bool(int(os.environ.get("KERNEL_DEBUG", "0"))))
    res = run_bass_kernel_spmd(_CACHED_NC, in_maps, list(range(NCORES)),
                               trace=bool(int(os.environ.get("KERNEL_TRACE", "0"))))
    out = np.asarray(res.results[0]["out"], np.float32).reshape(())
    kernel.last_results = res
    return out


# revision 16
# speedup vs baseline: 1.3129x; 1.1600x over previous
"""Trainium2 Bass kernel for nn_CombinedCriterionAE (retrieval 1-NN + losses).

Strategy (8 NeuronCores, SPMD), v6:
  - gt sharded along L (32768 -> 4096/core); every core holds all preds.
  - s = -dist^2 as a K=24 bf16-split matmul (tracks fp32 rounding to ~1e-6).
  - Per pred-tile (128 preds x 4096 gt), columns are paired (l, l+2048) and
    consumed in two steps s=0,1. Step s computes C(s) = cols [1024s, 1024s+1024)
    and C(s+2) = cols [2048+1024s, ...) as 4 concurrent 512-col matmuls on
    distinct PE row quadrants (tile_position): quadrants 0/1 -> Pa (= C(s)),
    quadrants 2/3 -> Pb (= C(s+2)). ONE ACT copy evacuates Pa to SBUF; the
    DVE scan (data0=Pb PSUM, data1=Pa-copy) emits the running max of pairs,
    chained across the two steps -> monotone SCN [128, 2048].
  - ONE ACT Sign+accum over the ODD scan positions (1024 strided reads):
    count c -> winner pair-position p* in {2c, 2c+1}. Candidate gt rows
    {a, a+1} u {a+2048, a+2049} with a = min(2c, 2046): two 2-consecutive-row
    indirect gathers per tile (descriptor-count bound, same cost as 1-row).
  - Sign counts are software-pipelined one tile behind so copies always
    precede signs in ACT program order (no scan-chain stalls).
  - PSUM: two pools (copied / scanned) of 2 banks x 2 bufs = 8 banks total,
    fully double-buffered.
  - Post-loop: exact fp32 d^2 over the 4 candidates folded in index order
    (first-occurrence ties match the reference), AllReduce-min over d2,
    equality-masked cos, scalar AllReduce-sum for the cos term.
"""
import os
import numpy as np
import ml_dtypes

import concourse.bass as bass
import concourse.bacc as bacc
import concourse.mybir as mybir
import concourse.tile as tile
from concourse.bass import IndirectOffsetOnAxis

BF16 = ml_dtypes.bfloat16
DT = mybir.dt
OP = mybir.AluOpType
AF = mybir.ActivationFunctionType

N_PRED = 8192
L_GT = 32768
NCORES = 8
K_SMALL = 19
K_BIG = 5
KK = K_SMALL + K_BIG
NEG_INF = -3.0e38
GRP = 8  # tiles per decode group
NLHS = 4  # LHS split for earlier loop start


# ----------------------------------------------------------------------------
# host-side input prep
# ----------------------------------------------------------------------------

def _split3(x):
    x = np.asarray(x, np.float32)
    hi = x.astype(BF16)
    r = x - hi.astype(np.float32)
    mid = r.astype(BF16)
    r2 = r - mid.astype(np.float32)
    lo = r2.astype(BF16)
    return hi, mid, lo


def build_operands(pred_pts, gt_pts):
    """lhsT [24, N] / rhs [24, L] bf16; 19 small rows then 5 big rows."""
    q = 2.0 * np.asarray(pred_pts, np.float32)
    qh, qm, ql = _split3(q.T)
    gh, gm, gl = _split3(np.asarray(gt_pts, np.float32).T)
    g2 = (np.asarray(gt_pts, np.float32) ** 2).sum(1)
    p2 = (np.asarray(pred_pts, np.float32) ** 2).sum(1)
    g2h, g2m, g2l = _split3(g2)
    p2h, p2m, p2l = _split3(p2)
    ones_g = np.ones(gt_pts.shape[0], BF16)
    neg1_p = -np.ones(pred_pts.shape[0], BF16)

    lhs, rhs = [], []

    def add(a, b):
        lhs.append(a)
        rhs.append(b)

    for d in range(3):
        add(qh[d], gm[d]); add(qm[d], gh[d]); add(qm[d], gm[d])
        add(qh[d], gl[d]); add(ql[d], gh[d])
    add(neg1_p, g2m); add(neg1_p, g2l)
    add((-p2m).astype(BF16), ones_g); add((-p2l).astype(BF16), ones_g)
    # big rows
    add(qh[0], gh[0]); add(qh[1], gh[1]); add(qh[2], gh[2])
    add((-p2h).astype(BF16), ones_g); add(neg1_p, g2h)
    return np.ascontiguousarray(np.stack(lhs)), np.ascontiguousarray(np.stack(rhs))


def prep_inputs(pred_feat, gt_data, n_pred, ll, ncores):
    """Returns the per-core in_map list."""
    pred_feat = np.asarray(pred_feat, np.float32)
    gt_data = np.asarray(gt_data, np.float32)
    nt = n_pred // 128
    pred_pts = pred_feat[:, :3]
    pred_nrm = pred_feat[:, 3:]
    lhsT, rhs = build_operands(pred_pts, gt_data[:, :3])

    # 4-way tile_position packing: row quadrant q holds the same 24 lhsT rows.
    lhs4 = np.zeros((128, n_pred), BF16)
    for q in range(4):
        lhs4[32 * q:32 * q + KK] = lhsT

    # pred arrays in [128, nt, 3] layout: element (r, i, :) = pred[i*128+r]
    pp = np.ascontiguousarray(pred_pts.reshape(nt, 128, 3).transpose(1, 0, 2))
    pn = np.ascontiguousarray(pred_nrm.reshape(nt, 128, 3).transpose(1, 0, 2))

    in_maps = []
    for c in range(ncores):
        rc = rhs[:, ll * c:ll * (c + 1)]
        # rhs4[32q+k, 512s+v]: quadrant q, step s:
        #   col0 = 2048*(q//2) + 1024*s + 512*(q%2)
        rhs4 = np.zeros((128, 1024), BF16)
        for q in range(4):
            for s in range(2):
                col0 = 2048 * (q // 2) + 1024 * s + 512 * (q % 2)
                rhs4[32 * q:32 * q + KK, 512 * s:512 * (s + 1)] = \
                    rc[:, col0:col0 + 512]
        in_maps.append({
            "rhs4": np.ascontiguousarray(rhs4),
            **{f"lhs4_{j}": np.ascontiguousarray(
                lhs4[:, j * (n_pred // NLHS):(j + 1) * (n_pred // NLHS)])
               for j in range(NLHS)},
            "pp": pp,
            "pn": pn,
            "cbase": np.full((128, 1), float(ll * c), np.float32),
            # paired rows: gtf[l] = concat(gt[l], gt[l+1]); one single-offset
            # 48B gather fetches both candidates of an adjacent pair
            "gtf": np.ascontiguousarray(np.concatenate(
                [gt_data, np.roll(gt_data, -1, axis=0)], axis=1)),
        })
    return in_maps


# ----------------------------------------------------------------------------
# device program
# ----------------------------------------------------------------------------

def build_nc(n_pred=N_PRED, ll=L_GT // NCORES, ncores=NCORES, debug_outs=False):
    nt = n_pred // 128
    assert ll == 4096 and n_pred % 128 == 0 and nt % GRP == 0
    l_tot = ll * ncores
    nsub = n_pred // NLHS
    tsub = nsub // 128  # pred-tiles per LHS subtile

    nc = bacc.Bacc("TRN2", target_bir_lowering=False, debug=False,
                   num_devices=ncores)

    lhs_d = [nc.dram_tensor(f"lhs4_{j}", [128, nsub], DT.bfloat16,
                            kind="ExternalInput") for j in range(NLHS)]
    rhs4_d = nc.dram_tensor("rhs4", [128, 1024], DT.bfloat16, kind="ExternalInput")
    pp_d = nc.dram_tensor("pp", [128, nt, 3], DT.float32, kind="ExternalInput")
    pn_d = nc.dram_tensor("pn", [128, nt, 3], DT.float32, kind="ExternalInput")
    cbase_d = nc.dram_tensor("cbase", [128, 1], DT.float32, kind="ExternalInput")
    gtf_d = nc.dram_tensor("gtf", [l_tot, 12], DT.float32, kind="ExternalInput")
    out_d = nc.dram_tensor("out", [1, 1], DT.float32, kind="ExternalOutput")
    if debug_outs:
        dbg_cnt_d = nc.dram_tensor("dbg_cnt", [128, nt], DT.float32, kind="ExternalOutput")
        dbg_i0_d = nc.dram_tensor("dbg_i0", [128, nt], DT.float32, kind="ExternalOutput")
        dbg_d2_d = nc.dram_tensor("dbg_d2", [128, nt], DT.float32, kind="ExternalOutput")
        dbg_cl_d = nc.dram_tensor("dbg_cl", [128, nt], DT.float32, kind="ExternalOutput")

    with tile.TileContext(nc) as tc:
        with (
            tc.tile_pool(name="persist", bufs=1) as pers,
            tc.tile_pool(name="hpool", bufs=4) as hpool,
            tc.tile_pool(name="scnpool", bufs=3) as scnpool,
            tc.tile_pool(name="mkpool", bufs=2) as mkpool,
            tc.tile_pool(name="dpool", bufs=3) as dpool,
            tc.tile_pool(name="dram", bufs=1, space="DRAM") as dram,
        ):
            # ---- persistent SBUF loads -------------------------------------
            RHS4 = pers.tile([128, 1024], DT.bfloat16)
            nc.sync.dma_start(RHS4[:], rhs4_d[:])
            LHS = []
            for j in range(NLHS):
                L = pers.tile([128, nsub], DT.bfloat16, name=f"LHS{j}")
                nc.sync.dma_start(L[:], lhs_d[j][:])
                LHS.append(L)
            PP = pers.tile([128, nt, 3], DT.float32)
            PN = pers.tile([128, nt, 3], DT.float32)
            CBASE = pers.tile([128, 1], DT.float32)
            nc.sync.dma_start(PP[:], pp_d[:])
            nc.sync.dma_start(PN[:], pn_d[:])
            nc.sync.dma_start(CBASE[:], cbase_d[:])

            CNT = pers.tile([128, nt], DT.float32)
            I0T = pers.tile([128, nt], DT.int32)
            I1T = pers.tile([128, nt], DT.int32)
            GA = pers.tile([128, nt, 2, 6], DT.float32)
            GB = pers.tile([128, nt, 2, 6], DT.float32)

            # ---- main loop (signs pipelined one tile behind) ----------------
            # psb (scanned chunks) gets 3 bufs so Pb(i+1) matmuls overlap
            # scan(i); psa (copied chunks) drains fast via ACT, 1 buf suffices
            with (
                tc.tile_pool(name="psa", bufs=1, space="PSUM") as psa,
                tc.tile_pool(name="psb", bufs=3, space="PSUM") as psb,
            ):
                scn_tiles = {}

                def emit_compute(i):
                    lhs_t = LHS[i // tsub]
                    lo = 128 * (i % tsub)
                    SCN = scnpool.tile([128, 2048], DT.float32, tag="SCN")
                    for s in range(2):
                        Pa = psa.tile([128, 1024], DT.float32, tag="Pa")
                        Pb = psb.tile([128, 1024], DT.float32, tag="Pb")
                        # Pb (scan-critical) matmuls first in PE issue order
                        for q in (2, 3, 0, 1):
                            P = Pa if q < 2 else Pb
                            h = q % 2
                            nc.tensor.matmul(
                                P[:, 512 * h:512 * (h + 1)],
                                lhs_t[32 * q:32 * q + KK, lo:lo + 128],
                                RHS4[32 * q:32 * q + KK, 512 * s:512 * (s + 1)],
                                start=True, stop=True,
                                tile_position=(32 * q, 0),
                            )
                        HB = hpool.tile([128, 1024], DT.float32, tag="HB")
                        nc.scalar.activation(out=HB[:], in_=Pa[:], func=AF.Copy)
                        nc.vector.tensor_tensor_scan(
                            out=SCN[:, 1024 * s:1024 * (s + 1)],
                            data0=Pb[:],
                            data1=HB[:],
                            initial=NEG_INF if s == 0 else SCN[:, 1023:1024],
                            op0=OP.max,
                            op1=OP.max,
                        )
                    scn_tiles[i] = SCN

                def emit_count(i):
                    SCN = scn_tiles.pop(i)
                    MK = mkpool.tile([128, 1024], DT.float16, tag="MK")
                    # count strided (odd) positions strictly below the max
                    nc.scalar.activation(
                        out=MK[:], in_=SCN[:, 1::2],
                        func=AF.Sign,
                        bias=SCN[:, 2047:2048],
                        scale=-1.0,
                        accum_out=CNT[:, i:i + 1],
                    )

                def emit_decode(g0, g1):
                    sl = slice(g0, g1)
                    n = g1 - g0
                    A = dpool.tile([128, GRP], DT.float32, tag="A")
                    B = dpool.tile([128, GRP], DT.float32, tag="B")
                    # a = min(2c, 2046) + cbase  (p* in {2c, 2c+1})
                    nc.vector.tensor_scalar(
                        out=A[:, :n], in0=CNT[:, sl],
                        scalar1=2.0, scalar2=2046.0, op0=OP.mult, op1=OP.min,
                    )
                    nc.vector.tensor_scalar(
                        out=A[:, :n], in0=A[:, :n],
                        scalar1=CBASE[:, 0:1], scalar2=None, op0=OP.add,
                    )
                    nc.vector.tensor_scalar(
                        out=B[:, :n], in0=A[:, :n],
                        scalar1=2048.0, scalar2=None, op0=OP.add,
                    )
                    nc.vector.tensor_copy(out=I0T[:, sl], in_=A[:, :n])
                    nc.vector.tensor_copy(out=I1T[:, sl], in_=B[:, :n])
                    for j in range(g0, g1):
                        nc.gpsimd.indirect_dma_start(
                            out=GA[:, j, :, :], out_offset=None, in_=gtf_d[:],
                            in_offset=IndirectOffsetOnAxis(ap=I0T[:, j:j + 1], axis=0),
                        )
                        nc.gpsimd.indirect_dma_start(
                            out=GB[:, j, :, :], out_offset=None, in_=gtf_d[:],
                            in_offset=IndirectOffsetOnAxis(ap=I1T[:, j:j + 1], axis=0),
                        )

                # stagger the trailing decode groups so only one tile's
                # gathers trail the last scan
                bounds = list(range(GRP, nt - GRP, GRP)) + [nt - 8, nt - 4, nt - 2, nt - 1]
                b_prev = 0
                for i in range(nt):
                    emit_compute(i)
                    if i >= 1:
                        emit_count(i - 1)
                        if i in bounds:
                            emit_decode(b_prev, i)
                            b_prev = i
                emit_count(nt - 1)
                emit_decode(b_prev, nt)

            # ---- exact fp32 resolve of the 4 candidates ---------------------
            # d2 min-fold first (feeds the collective ASAP); normals and cos
            # are computed while the AllReduce runs. Index order a < a+1 <
            # a+2048 < a+2049 with strict-less keeps first-occurrence ties.
            PPD = pers.tile([128, nt, 2, 3], DT.float32)
            nc.vector.tensor_copy(out=PPD[:, :, 0, :], in_=PP[:])
            nc.vector.tensor_copy(out=PPD[:, :, 1, :], in_=PP[:])
            DFA = pers.tile([128, nt, 2, 3], DT.float32)
            DA = pers.tile([128, nt, 2], DT.float32)
            DB = pers.tile([128, nt, 2], DT.float32)
            for G, D in ((GA, DA), (GB, DB)):
                nc.vector.tensor_tensor(out=DFA[:], in0=PPD[:], in1=G[:, :, :, 0:3],
                                        op=OP.subtract)
                nc.vector.tensor_tensor(out=DFA[:], in0=DFA[:], in1=DFA[:], op=OP.mult)
                nc.vector.tensor_reduce(out=D[:], in_=DFA[:],
                                        axis=mybir.AxisListType.X, op=OP.add)
            CMA = pers.tile([128, nt], DT.uint8)
            CMB = pers.tile([128, nt], DT.uint8)
            CMX = pers.tile([128, nt], DT.uint8)
            DAm = pers.tile([128, nt], DT.float32)
            DBm = pers.tile([128, nt], DT.float32)
            D2 = pers.tile([128, nt], DT.float32)
            nc.vector.tensor_tensor(out=CMA[:], in0=DA[:, :, 1], in1=DA[:, :, 0], op=OP.is_lt)
            nc.vector.tensor_tensor(out=DAm[:], in0=DA[:, :, 1], in1=DA[:, :, 0], op=OP.min)
            nc.vector.tensor_tensor(out=CMB[:], in0=DB[:, :, 1], in1=DB[:, :, 0], op=OP.is_lt)
            nc.vector.tensor_tensor(out=DBm[:], in0=DB[:, :, 1], in1=DB[:, :, 0], op=OP.min)
            nc.vector.tensor_tensor(out=CMX[:], in0=DBm[:], in1=DAm[:], op=OP.is_lt)
            nc.vector.tensor_tensor(out=D2[:], in0=DBm[:], in1=DAm[:], op=OP.min)

            # ---- AllReduce-min over d2 (starts while normals/cos compute) ---
            cc_in = dram.tile([128, nt], DT.float32)
            cc_out = dram.tile([128, nt], DT.float32, addr_space="Shared")
            nc.sync.dma_start(cc_in[:], D2[:])
            nc.gpsimd.collective_compute(
                "AllReduce",
                OP.min,
                replica_groups=[list(range(ncores))],
                ins=[cc_in[:].opt()],
                outs=[cc_out[:].opt()],
            )

            # matched normal via the fold masks (per-dim: mask free dims must
            # match data free dims)
            NA = pers.tile([128, nt, 3], DT.float32)
            NB = pers.tile([128, nt, 3], DT.float32)
            MNR = pers.tile([128, nt, 3], DT.float32)
            for dd in range(3):
                nc.vector.select(out=NA[:, :, dd], mask=CMA[:],
                                 on_true=GA[:, :, 1, 3 + dd], on_false=GA[:, :, 0, 3 + dd])
                nc.vector.select(out=NB[:, :, dd], mask=CMB[:],
                                 on_true=GB[:, :, 1, 3 + dd], on_false=GB[:, :, 0, 3 + dd])
                nc.vector.select(out=MNR[:, :, dd], mask=CMX[:],
                                 on_true=NB[:, :, dd], on_false=NA[:, :, dd])

            # ---- per-pred cos similarity -----------------------------------
            def normalize(src3, dst3, tagp):
                NSQ = pers.tile([128, nt, 3], DT.float32, tag=f"NSQ{tagp}", name=f"NSQ{tagp}")
                NS = pers.tile([128, nt], DT.float32, tag=f"NS{tagp}", name=f"NS{tagp}")
                nc.vector.tensor_tensor(out=NSQ[:], in0=src3, in1=src3, op=OP.mult)
                nc.vector.tensor_reduce(out=NS[:], in_=NSQ[:], axis=mybir.AxisListType.X, op=OP.add)
                nc.scalar.activation(out=NS[:], in_=NS[:], func=AF.Sqrt)
                nc.vector.tensor_scalar(out=NS[:], in0=NS[:], scalar1=1e-4,
                                        scalar2=None, op0=OP.max)
                nc.vector.reciprocal(out=NS[:], in_=NS[:])
                for d in range(3):
                    nc.vector.tensor_tensor(out=dst3[:, :, d], in0=src3[:, :, d],
                                            in1=NS[:], op=OP.mult)

            PNH = pers.tile([128, nt, 3], DT.float32)
            MNH = pers.tile([128, nt, 3], DT.float32)
            normalize(PN[:], PNH, "a")
            normalize(MNR[:], MNH, "b")
            CC3 = pers.tile([128, nt, 3], DT.float32)
            CL = pers.tile([128, nt], DT.float32)
            nc.vector.tensor_tensor(out=CC3[:], in0=PNH[:], in1=MNH[:], op=OP.mult)
            nc.vector.tensor_reduce(out=CL[:], in_=CC3[:], axis=mybir.AxisListType.X, op=OP.add)

            # ---- equality-masked cos against the global min -----------------
            D2G = pers.tile([128, nt], DT.float32)
            nc.sync.dma_start(D2G[:], cc_out[:])
            MSK = pers.tile([128, nt], DT.uint8)
            CLM = pers.tile([128, nt], DT.float32)
            nc.vector.tensor_tensor(out=MSK[:], in0=D2[:], in1=D2G[:], op=OP.is_equal)
            nc.vector.memset(CLM[:], 0.0)
            nc.vector.copy_predicated(out=CLM[:], mask=MSK[:], data=CL[:])
            if debug_outs:
                nc.sync.dma_start(dbg_cnt_d[:], CNT[:])
                DBG_I0 = pers.tile([128, nt], DT.float32)
                nc.vector.tensor_copy(out=DBG_I0[:], in_=GA[:, :, 1, 0])
                nc.sync.dma_start(dbg_i0_d[:], DBG_I0[:])
                nc.sync.dma_start(dbg_d2_d[:], D2[:])
                nc.sync.dma_start(dbg_cl_d[:], D2G[:])

            # ---- scalar losses ---------------------------------------------
            ILS = pers.tile([128, 1], DT.float32)
            CSUM = pers.tile([128, 1], DT.float32)
            nc.vector.tensor_reduce(out=ILS[:], in_=D2G[:],
                                    axis=mybir.AxisListType.X, op=OP.add)
            nc.vector.tensor_reduce(out=CSUM[:], in_=CLM[:],
                                    axis=mybir.AxisListType.X, op=OP.add)

            # partition-sum via ones-matmul
            SUM2 = pers.tile([128, 2], DT.float32)
            ONES = pers.tile([128, 1], DT.float32)
            nc.vector.memset(ONES[:], 1.0)
            nc.vector.tensor_copy(out=SUM2[:, 0:1], in_=ILS[:])
            nc.vector.tensor_copy(out=SUM2[:, 1:2], in_=CSUM[:])
            with tc.tile_pool(name="fpsum", bufs=1, space="PSUM") as fpsum:
                SP = fpsum.tile([1, 2], DT.float32)
                nc.tensor.matmul(SP[:], ONES[:], SUM2[:], start=True, stop=True)
                FIN = pers.tile([1, 2], DT.float32)
                nc.vector.tensor_copy(out=FIN[:], in_=SP[:])

            # cos partial sums must be AllReduce-summed (the inlier sum is
            # already globally identical after the min-AllReduce)
            cs_in = dram.tile([1, 1], DT.float32)
            cs_out = dram.tile([1, 1], DT.float32, addr_space="Shared")
            nc.sync.dma_start(cs_in[:], FIN[0:1, 1:2])
            nc.gpsimd.collective_compute(
                "AllReduce",
                OP.add,
                replica_groups=[list(range(ncores))],
                ins=[cs_in[:].opt()],
                outs=[cs_out[:].opt()],
            )
            CSG = pers.tile([1, 1], DT.float32)
            nc.sync.dma_start(CSG[:], cs_out[:])

            A1 = pers.tile([1, 1], DT.float32)
            B1 = pers.tile([1, 1], DT.float32)
            OUTS = pers.tile([1, 1], DT.float32)
            nc.vector.tensor_scalar(out=A1[:], in0=FIN[0:1, 0:1],
                                    scalar1=1.0 / (n_pred * 3), scalar2=None, op0=OP.mult)
            nc.vector.tensor_scalar(out=B1[:], in0=CSG[:],
                                    scalar1=1.0 / n_pred, scalar2=None, op0=OP.mult)
            nc.vector.tensor_tensor(out=OUTS[:], in0=A1[:], in1=B1[:], op=OP.subtract)
            nc.vector.tensor_scalar(out=OUTS[:], in0=OUTS[:], scalar1=1.0,
                                    scalar2=None, op0=OP.add)
            nc.sync.dma_start(out_d[:], OUTS[:])

    nc.compile()
    return nc


# ----------------------------------------------------------------------------
# public entry point
# ----------------------------------------------------------------------------

_CACHED_NC = None


def kernel(pred_feat, pred_decoder, input_data, gt_data):
    global _CACHED_NC
    from concourse.bass_utils import run_bass_kernel_spmd

    ll = L_GT // NCORES
    in_maps = prep_inputs(pred_feat, gt_data, N_PRED, ll, NCORES)
    if _CACHED_NC is None:
        _CACHED_NC = build_nc(N_PRED, ll, NCORES,
                              debug_outs=bool(int(os.environ.get("KERNEL_DEBUG", "0"))))
    res = run_bass_kernel_spmd(_CACHED_NC, in_maps, list(range(NCORES)),
                               trace=bool(int(os.environ.get("KERNEL_TRACE", "0"))))
    out = np.asarray(res.results[0]["out"], np.float32).reshape(())
    kernel.last_results = res
    return out
